# revision 8
# baseline (speedup 1.0000x reference)
"""Trainium2 Bass kernel for nn_MEModule (gnn_message_passing).

Math per edge e (reference):
    h_emb = [h[idx_s[e]], h[idx_t[e]]]                 # [24]
    a     = h_emb @ w1cat + b1cat                      # [72]  (w1cat[d,(m,f)] = w1[m,d,f])
    g     = h_emb @ w2cat + b2cat                      # [72]
    glu   = a * sigmoid(g)                             # [72]
    stk   = glu * rbf3          (rbf3[(m,d)] = rbf[d]) # [72]
    out   = stk @ wl + bl                              # [128]

Device layout ("T-layout"): edges on the free dim, features on partitions.
Host pre-gathers h_emb, pre-transposes, and interleaves with rbf into one
stream hr = [h_embT; rbf_T] of shape [48, E]; output is produced as
[128, E] and de-transposed on the host.  8-way edge sharding; no
collectives.  All weights travel in one packed [128, 347] tensor so every
matmul depends on a single weight-DMA semaphore.

Per 500-edge chunk on device:
    a_ps   = w1cat.T @ h_embT          (PE, PSUM [72,500])
    g_ps   = w2cat.T @ h_embT          (PE)
    r_ps   = brep.T  @ rbf_T           (PE; brep = [I24 I24 I24] replicates rbf)
    sig    = sigmoid(g_ps + b2cat)     (ACT, bias = per-partition AP)
    glu    = (a_ps + b1cat) * sig      (DVE scalar_tensor_tensor)
    stk    = glu * r_ps                (DVE tensor_mul)
    o_ps   = wl.T @ stk                (PE, PSUM [128,500])
    out    = o_ps + blcat              (ACT Identity w/ bias -> SBUF, DMA out)
"""

import numpy as np

N_CORES = 8
E_TOTAL = 2_000_000
EMB = 12
D = 24            # 2*EMB
HR = 2 * D        # 48: h_embT rows + rbf_T rows
KF = 72           # NUM_MODULES * D
OUT = 128
SUPER = 5000      # edges per DMA supertile
CHUNK = 500       # edges per PSUM chunk (matmul N, <=512 fp32)

# packed-weights column layout ([128, WP_F] tensor)
W1_C, W2_C, BR_C, WL_C = 0, 72, 144, 216
B1_C, B2_C, BL_C = 344, 345, 346
WP_F = 347


def build_nc(e_shard: int, super_: int = SUPER, chunk: int = CHUNK):
    from contextlib import ExitStack

    import concourse.tile as tile
    from concourse import bacc, mybir

    f32 = mybir.dt.float32
    assert e_shard % super_ == 0 and super_ % chunk == 0
    n_super = e_shard // super_
    n_chunk = super_ // chunk

    try:
        from concourse._compat import get_trn_type
        trn = get_trn_type() or "TRN2"
    except Exception:
        trn = "TRN2"
    nc = bacc.Bacc(trn, target_bir_lowering=False, debug=False)
    hr = nc.declare_dram_parameter("hr", [D, 2 * e_shard], f32, isOutput=False)
    wpk = nc.declare_dram_parameter("wpack", [OUT, WP_F], f32, isOutput=False)
    outT = nc.declare_dram_parameter("outT", [OUT, e_shard], f32, isOutput=True)

    with ExitStack() as ctx:
        tc = ctx.enter_context(tile.TileContext(nc))
        wpool = ctx.enter_context(tc.tile_pool(name="weights", bufs=1))
        sb = ctx.enter_context(tc.tile_pool(name="sbuf", bufs=2))
        vb = ctx.enter_context(tc.tile_pool(name="vecbuf", bufs=2))
        ps = ctx.enter_context(tc.tile_pool(name="psum", bufs=2, space="PSUM"))

        wp = wpool.tile([OUT, WP_F], f32, tag="wp")
        nc.sync.dma_start(out=wp[:], in_=wpk[:])
        w1_t = wp[0:D, W1_C : W1_C + KF]
        w2_t = wp[0:D, W2_C : W2_C + KF]
        br_t = wp[0:D, BR_C : BR_C + KF]
        wl_t = wp[0:KF, WL_C : WL_C + OUT]
        b1_t = wp[0:KF, B1_C : B1_C + 1]
        b2_t = wp[0:KF, B2_C : B2_C + 1]
        bl_t = wp[0:OUT, BL_C : BL_C + 1]

        for st in range(n_super):
            s0 = st * super_
            hrt = sb.tile([D, 2 * super_], f32, tag="hrt")
            ot = sb.tile([OUT, super_], f32, tag="ot")
            nc.sync.dma_start(out=hrt[:], in_=hr[:, 2 * s0 : 2 * s0 + 2 * super_])
            for c in range(n_chunk):
                sl = slice(c * chunk, (c + 1) * chunk)
                ht = hrt[0:D, c * chunk : (c + 1) * chunk]
                rt = hrt[0:D, super_ + c * chunk : super_ + (c + 1) * chunk]
                a_ps = ps.tile([KF, chunk], f32, tag="a")
                g_ps = ps.tile([KF, chunk], f32, tag="g")
                r_ps = ps.tile([KF, chunk], f32, tag="r")
                o_ps = ps.tile([OUT, chunk], f32, tag="o")
                nc.tensor.matmul(out=a_ps[:], lhsT=w1_t, rhs=ht,
                                 start=True, stop=True)
                nc.tensor.matmul(out=g_ps[:], lhsT=w2_t, rhs=ht,
                                 start=True, stop=True)
                nc.tensor.matmul(out=r_ps[:], lhsT=br_t, rhs=rt,
                                 start=True, stop=True)
                sig = vb.tile([KF, chunk], f32, tag="sig")
                nc.scalar.activation(out=sig[:], in_=g_ps[:],
                                     func=mybir.ActivationFunctionType.Sigmoid,
                                     bias=b2_t, scale=1.0)
                glu = vb.tile([KF, chunk], f32, tag="glu")
                nc.vector.scalar_tensor_tensor(out=glu[:], in0=a_ps[:],
                                               scalar=b1_t, in1=sig[:],
                                               op0=mybir.AluOpType.add,
                                               op1=mybir.AluOpType.mult)
                stk = vb.tile([KF, chunk], f32, tag="stk")
                nc.vector.tensor_mul(out=stk[:], in0=glu[:], in1=r_ps[:])
                nc.tensor.matmul(out=o_ps[:], lhsT=wl_t, rhs=stk[:],
                                 start=True, stop=True)
                nc.scalar.activation(out=ot[:, sl], in_=o_ps[:],
                                     func=mybir.ActivationFunctionType.Identity,
                                     bias=bl_t, scale=1.0)
            nc.sync.dma_start(out=outT[:, s0 : s0 + super_], in_=ot[:])
    nc.compile()
    return nc


def pack_weights(w1, b1, w2, b2, wl, bl):
    wp = np.zeros((OUT, WP_F), dtype=np.float32)
    w1cat = np.asarray(w1, np.float32).transpose(1, 0, 2).reshape(D, KF)
    w2cat = np.asarray(w2, np.float32).transpose(1, 0, 2).reshape(D, KF)
    brep = np.concatenate([np.eye(D, dtype=np.float32)] * 3, axis=1)
    wp[0:D, W1_C : W1_C + KF] = w1cat
    wp[0:D, W2_C : W2_C + KF] = w2cat
    wp[0:D, BR_C : BR_C + KF] = brep
    wp[0:KF, WL_C : WL_C + OUT] = np.asarray(wl, np.float32)
    wp[0:KF, B1_C] = np.asarray(b1, np.float32).reshape(KF)
    wp[0:KF, B2_C] = np.asarray(b2, np.float32).reshape(KF)
    wp[0:OUT, BL_C] = np.asarray(bl, np.float32).reshape(OUT)
    return wp


def prep_inputs(rbf, h, idx_s, idx_t, w1, b1, w2, b2, wl, bl,
                e_total=E_TOTAL, n_cores=N_CORES):
    """Host-side marshaling: gather, transpose, shard."""
    rbf = np.asarray(rbf, dtype=np.float32)
    h = np.asarray(h, dtype=np.float32)
    idx_s = np.asarray(idx_s).astype(np.int64)
    idx_t = np.asarray(idx_t).astype(np.int64)
    ec = e_total // n_cores

    # Per-supertile interleave: hr[:, 2*s0 : 2*s0+S] = h_embT block,
    # hr[:, 2*s0+S : 2*s0+2S] = rbf_T block, so the device loads one
    # [24, 2S] tile per supertile with both operands at base partition 0.
    hembT = np.empty((D, e_total), dtype=np.float32)
    hembT[0:EMB, :] = h[idx_s].T
    hembT[EMB:D, :] = h[idx_t].T
    rbfT = rbf.T
    n_super = ec // SUPER
    wp = pack_weights(w1, b1, w2, b2, wl, bl)
    in_maps = []
    for i in range(n_cores):
        s = slice(i * ec, (i + 1) * ec)
        hb = hembT[:, s].reshape(D, n_super, SUPER)
        rb = rbfT[:, s].reshape(D, n_super, SUPER)
        hr = np.ascontiguousarray(
            np.stack([hb, rb], axis=2).reshape(D, 2 * ec))
        in_maps.append({"hr": hr, "wpack": wp})
    return in_maps


def build_exec(nc, in_maps):
    """Mirror bass2jax.run_bass_via_pjrt but stage inputs on device once and
    return (fn, dev_args, assemble) so callers can time pure execution."""
    import jax
    import jax.numpy as jnp
    from jax.sharding import Mesh, PartitionSpec, NamedSharding
    from jax.experimental.shard_map import shard_map
    import concourse.mybir as mybir
    from concourse.bass2jax import (_bass_exec_p, install_neuronx_cc_hook,
                                    partition_id_tensor)

    install_neuronx_cc_hook()
    n_cores = len(in_maps)
    in_names, out_names, out_avals = [], [], []
    partition_name = (nc.partition_id_tensor.name
                      if nc.partition_id_tensor else None)
    for alloc in nc.m.functions[0].allocations:
        if not isinstance(alloc, mybir.MemoryLocationSet):
            continue
        name = alloc.memorylocations[0].name
        if alloc.kind == "ExternalInput":
            if name != partition_name:
                in_names.append(name)
        elif alloc.kind == "ExternalOutput":
            out_names.append(name)
            out_avals.append(jax.core.ShapedArray(
                tuple(alloc.tensor_shape), mybir.dt.np(alloc.dtype)))
    n_params = len(in_names)
    all_in_names = list(in_names) + list(out_names)
    if partition_name is not None:
        all_in_names.append(partition_name)

    def _body(*args):
        operands = list(args)
        if partition_name is not None:
            operands.append(partition_id_tensor())
        return tuple(_bass_exec_p.bind(
            *operands,
            out_avals=tuple(out_avals),
            in_names=tuple(all_in_names),
            out_names=tuple(out_names),
            lowering_input_output_aliases=(),
            sim_require_finite=True,
            sim_require_nnan=True,
            nc=nc,
        ))

    devices = jax.devices()[:n_cores]
    mesh = Mesh(np.asarray(devices), ("core",))
    n_outs = len(out_names)
    in_specs = (PartitionSpec("core"),) * (n_params + n_outs)
    out_specs = (PartitionSpec("core"),) * n_outs
    fn = jax.jit(shard_map(_body, mesh=mesh, in_specs=in_specs,
                           out_specs=out_specs, check_rep=False),
                 keep_unused=True)
    sh = NamedSharding(mesh, PartitionSpec("core"))
    dev_args = []
    for i, name in enumerate(in_names):
        cat = np.concatenate([np.asarray(m[name]) for m in in_maps], axis=0)
        dev_args.append(jax.device_put(cat, sh))
    for av in out_avals:
        z = jnp.zeros((n_cores * av.shape[0], *av.shape[1:]), av.dtype)
        dev_args.append(jax.device_put(z, sh))

    def assemble(out_arrs):
        res = []
        for c in range(n_cores):
            res.append({name: np.asarray(out_arrs[i]).reshape(
                n_cores, *out_avals[i].shape)[c]
                for i, name in enumerate(out_names)})
        return res

    return fn, dev_args, assemble


def run(rbf, h, idx_s, idx_t, w1, b1, w2, b2, wl, bl, time_iters=0):
    import time as _time

    e_total = rbf.shape[0]
    ec = e_total // N_CORES
    in_maps = prep_inputs(rbf, h, idx_s, idx_t, w1, b1, w2, b2, wl, bl,
                          e_total=e_total)
    nc = build_nc(ec)
    fn, dev_args, assemble = build_exec(nc, in_maps)
    out_arrs = fn(*dev_args)  # compile + first run
    import jax
    jax.block_until_ready(out_arrs)
    times = []
    for _ in range(time_iters):
        t0 = _time.perf_counter()
        jax.block_until_ready(fn(*dev_args))
        times.append(_time.perf_counter() - t0)
    results = assemble(out_arrs)
    out = np.empty((e_total, OUT), dtype=np.float32)
    for i in range(N_CORES):
        out[i * ec : (i + 1) * ec] = results[i]["outT"].T
    return out, times


def kernel(rbf, h, idx_s, idx_t, w1, b1, w2, b2, wl, bl):
    out, _ = run(rbf, h, idx_s, idx_t, w1, b1, w2, b2, wl, bl)
    return out
